# revision 2
# baseline (speedup 1.0000x reference)
"""Trainium2 Bass kernel for an RNN-T style joint network MLP.

  out[b,t,u,o] = tanh(enc[b,t,:] @ W1[:512] + dec[b,u,:] @ W1[512:] + b1) @ W2 + b2

Shapes: enc (8, 256, 512), dec (8, 64, 512), W1 (1024, 1024), b1 (1024,),
W2 (1024, 128), b2 (128,), out (8, 256, 64, 128), all float32.

Sharding: data-parallel over batch - one batch element per NeuronCore, no
collectives. The kernel is ACT-bound (16.8M tanh/core at 1 elem/cycle/lane
@1.2GHz = 109us floor), so the whole design keeps ACT 100% busy on nothing
but 16 full-width tanh ops:
  - ACT: one [128, 8192] tanh per u-block (quarter-split on blk 0 so it can
    start as soon as 2 h-chunks of the first GEMM land; pair-split on the
    last block so PE/DVE/DMA can chase the drain).
  - DVE: bias-broadcast adds sum[h,u,t] = e_proj[h,t] + bias[h,u]
    (tensor_scalar, 2x mode, ~196ns per [128,256]) + PSUM evac with +b2.
  - GPSIMD: takes 8 of the 32 adds per block (hc 6-7) so DVE stays under
    the 7.0us ACT pace; also issues input DMAs (cheap software DGE).
  - PE: first GEMMs + main GEMM (bf16, ~216ns per N=512 matmul) + ~3us of
    junk warmup matmuls at t=0 so the HAM clock-gate is at 2.4GHz before
    the real GEMMs arrive.
Inputs are host-rearranged so every DMA is contiguous, W1 is hc-major and
split into per-hc DMAs across 3 queues so the first quarter's weights land
in ~2us.
"""

import os
import numpy as np
import ml_dtypes

B, T, U, D, H, O = 8, 256, 64, 512, 1024, 128
NCORES = 8
HC = H // 128      # 8 h-chunks
UB = 4             # u-block size
NB = U // UB       # 16 blocks
BW = UB * T * HC   # 8192 per-block sum/tanh width (hc-major: [hc][u][t])
WARM_MMS = 26      # ~3us of junk matmuls to flip HAM to 2.4GHz

_CACHE = {}
LAST_RESULT = None


def _build_program():
    from concourse import bacc, tile
    import concourse.mybir as mybir

    dt = mybir.dt
    f32, bf16 = dt.float32, dt.bfloat16
    Act = mybir.ActivationFunctionType

    nc = bacc.Bacc("TRN2", target_bir_lowering=False, debug=False)

    encT = nc.dram_tensor("encT", [128, 4 * T], bf16, kind="ExternalInput").ap()
    decT = nc.dram_tensor("decT", [128, 4 * U], bf16, kind="ExternalInput").ap()
    w1r = nc.dram_tensor("w1r", [128, HC * 1024], bf16, kind="ExternalInput").ap()
    w2r = nc.dram_tensor("w2r", [128, HC * O], bf16, kind="ExternalInput").ap()
    b1r = nc.dram_tensor("b1r", [128, HC], f32, kind="ExternalInput").ap()
    b2c = nc.dram_tensor("b2c", [O, 1], f32, kind="ExternalInput").ap()
    outT = nc.dram_tensor("outT", [O, U, T], f32, kind="ExternalOutput").ap()

    with tile.TileContext(nc) as tc:
        with tc.tile_pool(name="persist", bufs=1) as persist, \
             tc.tile_pool(name="sums", bufs=3) as sums_pool, \
             tc.tile_pool(name="tanhp", bufs=3) as tanh_pool, \
             tc.tile_pool(name="outsb", bufs=6) as out_pool, \
             tc.tile_pool(name="psum", bufs=8, space="PSUM") as psum_pool:

            w1_sb = persist.tile([128, HC * 1024], bf16, tag="w1")
            encT_sb = persist.tile([128, 4 * T], bf16, tag="encT")
            decT_sb = persist.tile([128, 4 * U], bf16, tag="decT")
            w2_sb = persist.tile([128, HC * O], bf16, tag="w2")
            b1_sb = persist.tile([128, HC], f32, tag="b1")
            b2_sb = persist.tile([128, 1], f32, tag="b2")
            e_sb = persist.tile([128, HC * T], bf16, tag="eproj")
            bias_sb = persist.tile([128, HC * U], f32, tag="bias")
            junk_w = persist.tile([128, 128], bf16, tag="junkw")
            junk_r = persist.tile([128, 128], bf16, tag="junkr")
            warm_a = persist.tile([128, 128], bf16, tag="warma")
            warm_o = persist.tile([128, 128], bf16, tag="warmo")

            # --- t=0: engine warmups --------------------------------------
            # GPSIMD memsets feed the PE junk matmuls (no DMA dependency).
            nc.gpsimd.memset(junk_w[:], 0.0)
            nc.gpsimd.memset(junk_r[:], 0.0)
            nc.gpsimd.memset(warm_a[:], 0.0)
            # ACT: dummy tanh pulls the ~2.7us ACT_TABLE_LOAD off the
            # critical path (overlaps the input DMAs).
            nc.scalar.activation(warm_o[:], warm_a[:], Act.Tanh)
            # PE: junk matmuls keep the HAM activity window busy so the
            # real first GEMM runs at 2.4GHz instead of 1.2.
            for i in range(WARM_MMS):
                jp = psum_pool.tile([128, 128], f32, tag="ps", name=f"junk{i}")
                nc.tensor.matmul(jp[:], lhsT=junk_w[:], rhs=junk_r[:],
                                 start=True, stop=True)

            # --- input DMAs, 3 queues, earliest-needed first --------------
            # gpsimd queue (software DGE, ~25ns issue): head-critical loads
            nc.gpsimd.dma_start(b1_sb[:], b1r[:, :])
            nc.gpsimd.dma_start(decT_sb[:], decT[:, :])
            for hc in (0, 1, 2, 6, 7):
                nc.gpsimd.dma_start(w1_sb[:, hc * 1024:(hc + 1) * 1024],
                                    w1r[:, hc * 1024:(hc + 1) * 1024])
            # sync queue: enc + mid W1 chunks
            nc.sync.dma_start(encT_sb[:], encT[:, :])
            for hc in (3, 4, 5):
                nc.sync.dma_start(w1_sb[:, hc * 1024:(hc + 1) * 1024],
                                  w1r[:, hc * 1024:(hc + 1) * 1024])
            # scalar queue: W2/b2 (needed only when the first main GEMM runs)
            nc.scalar.dma_start(w2_sb[:], w2r[:, :])
            nc.scalar.dma_start(b2_sb[:], b2c[:, :])

            # --- first GEMMs per h-chunk (dec first: its data lands first)
            # w1 layout per hc: dcs 0-3 enc chunks, 4-7 dec chunks.
            for hc in range(HC):
                pd = psum_pool.tile([128, U], f32, tag="ps", name=f"pd{hc}")
                for dc in range(4):
                    nc.tensor.matmul(
                        pd[:],
                        lhsT=w1_sb[:, hc * 1024 + (4 + dc) * 128:
                                   hc * 1024 + (4 + dc) * 128 + 128],
                        rhs=decT_sb[:, dc * U:(dc + 1) * U],
                        start=(dc == 0), stop=(dc == 3),
                    )
                nc.vector.tensor_scalar_add(bias_sb[:, hc * U:(hc + 1) * U],
                                            pd[:], b1_sb[:, hc:hc + 1])

                pe = psum_pool.tile([128, T], f32, tag="ps", name=f"pe{hc}")
                for dc in range(4):
                    nc.tensor.matmul(
                        pe[:],
                        lhsT=w1_sb[:, hc * 1024 + dc * 128:
                                   hc * 1024 + dc * 128 + 128],
                        rhs=encT_sb[:, dc * T:(dc + 1) * T],
                        start=(dc == 0), stop=(dc == 3),
                    )
                nc.vector.tensor_copy(e_sb[:, hc * T:(hc + 1) * T], pe[:])

            # --- steady loop over u-blocks --------------------------------
            for blk in range(NB):
                sum_sb = sums_pool.tile([128, BW], bf16, tag="sum")
                for hc in range(HC):
                    for ul in range(UB):
                        u = blk * UB + ul
                        dst = sum_sb[:, hc * (UB * T) + ul * T:
                                     hc * (UB * T) + ul * T + T]
                        src = e_sb[:, hc * T:(hc + 1) * T]
                        sc = bias_sb[:, hc * U + u: hc * U + u + 1]
                        # GPSIMD takes hc 6-7 (blk>=1) to keep DVE under the
                        # ACT pace; blk 0 stays on DVE so the head quarter
                        # ordering isn't gated on slow GPSIMD ops.
                        if hc >= 6 and blk >= 1:
                            nc.gpsimd.tensor_scalar_add(dst, src, sc)
                        else:
                            nc.vector.tensor_scalar_add(dst, src, sc)

                tanh_sb = tanh_pool.tile([128, BW], bf16, tag="tanh")
                if blk == 0:
                    # quarter-split: quarter q needs only h-chunks 2q,2q+1
                    for q in range(4):
                        nc.scalar.activation(
                            tanh_sb[:, q * BW // 4:(q + 1) * BW // 4],
                            sum_sb[:, q * BW // 4:(q + 1) * BW // 4], Act.Tanh)
                elif blk == NB - 1:
                    # pair-split along u so the drain can be chased: strided
                    # 3D AP [128, hc=8, 512] per u-pair
                    sv = sum_sb[:, :].rearrange("p (c x) -> p c x", c=HC)
                    tv = tanh_sb[:, :].rearrange("p (c x) -> p c x", c=HC)
                    for p in range(2):
                        nc.scalar.activation(tv[:, :, p * 512:(p + 1) * 512],
                                             sv[:, :, p * 512:(p + 1) * 512],
                                             Act.Tanh)
                else:
                    nc.scalar.activation(tanh_sb[:], sum_sb[:], Act.Tanh)

                # main GEMM: psum[o, pair cols] = sum_hc W2[hc]^T @ tanh[hc]
                pos = [psum_pool.tile([128, 512], f32, tag="ps",
                                      name=f"po{blk}_{p}") for p in range(2)]
                if blk == NB - 1:
                    # pair-outer so pair 0 finishes right after its tanh
                    for p in range(2):
                        for hc in range(HC):
                            nc.tensor.matmul(
                                pos[p][:],
                                lhsT=w2_sb[:, hc * O:(hc + 1) * O],
                                rhs=tanh_sb[:, hc * (UB * T) + p * 512:
                                            hc * (UB * T) + (p + 1) * 512],
                                start=(hc == 0), stop=(hc == HC - 1),
                            )
                        osb = out_pool.tile([128, 512], f32, tag="osb",
                                            name=f"ot{p}")
                        nc.vector.tensor_scalar_add(osb[:], pos[p][:],
                                                    b2_sb[:, 0:1])
                        nc.sync.dma_start(
                            outT[:, blk * UB + 2 * p: blk * UB + 2 * p + 2, :],
                            osb[:])
                else:
                    # hc-outer: W2 chunk stays stationary across both pairs
                    for hc in range(HC):
                        for p in range(2):
                            nc.tensor.matmul(
                                pos[p][:],
                                lhsT=w2_sb[:, hc * O:(hc + 1) * O],
                                rhs=tanh_sb[:, hc * (UB * T) + p * 512:
                                            hc * (UB * T) + (p + 1) * 512],
                                start=(hc == 0), stop=(hc == HC - 1),
                            )
                    for p in range(2):
                        osb = out_pool.tile([128, 512], f32, tag="osb",
                                            name=f"o{blk}_{p}")
                        nc.vector.tensor_scalar_add(osb[:], pos[p][:],
                                                    b2_sb[:, 0:1])
                        nc.sync.dma_start(
                            outT[:, blk * UB + 2 * p: blk * UB + 2 * p + 2, :],
                            osb[:])

    nc.compile()
    return nc


def kernel(encoder_state, decoder_state, W1, b1, W2, b2):
    from concourse.bass_utils import run_bass_kernel_spmd
    global LAST_RESULT

    if "nc" not in _CACHE:
        _CACHE["nc"] = _build_program()
    nc = _CACHE["nc"]

    encoder_state = np.asarray(encoder_state, dtype=np.float32)
    decoder_state = np.asarray(decoder_state, dtype=np.float32)
    W1 = np.asarray(W1, dtype=np.float32)
    b1 = np.asarray(b1, dtype=np.float32)
    W2 = np.asarray(W2, dtype=np.float32)
    b2 = np.asarray(b2, dtype=np.float32)

    bf = ml_dtypes.bfloat16
    # W1 [2D, H] -> hc-major [128, hc(8), dcs(8), 128]: per hc, 4 enc then
    # 4 dec contraction chunks, each [128 x 128] with d on partitions.
    W1r = np.ascontiguousarray(
        W1.astype(bf).reshape(8, 128, 8, 128).transpose(1, 2, 0, 3)
        .reshape(128, 8192))
    # W2 [H, O] -> [128, hc(8)*O] with h-within-chunk on partitions
    W2r = np.ascontiguousarray(
        W2.astype(bf).reshape(8, 128, O).transpose(1, 0, 2).reshape(128, 8 * O))
    b1r = np.ascontiguousarray(b1.reshape(HC, 128).T)
    b2c = np.ascontiguousarray(b2.reshape(O, 1))

    in_maps = []
    for i in range(NCORES):
        encT = np.ascontiguousarray(
            encoder_state[i].T.astype(bf).reshape(4, 128, T)
            .transpose(1, 0, 2).reshape(128, 4 * T))
        decT = np.ascontiguousarray(
            decoder_state[i].T.astype(bf).reshape(4, 128, U)
            .transpose(1, 0, 2).reshape(128, 4 * U))
        in_maps.append({
            "encT": encT,
            "decT": decT,
            "w1r": W1r,
            "w2r": W2r,
            "b1r": b1r,
            "b2c": b2c,
        })

    trace = bool(int(os.environ.get("KERNEL_TRACE", "0")))
    res = run_bass_kernel_spmd(nc, in_maps, list(range(NCORES)), trace=trace)
    LAST_RESULT = res

    out = np.empty((B, T, U, O), dtype=np.float32)
    for i in range(NCORES):
        out[i] = res.results[i]["outT"].transpose(2, 1, 0)
    return out


# revision 3
# speedup vs baseline: 3.5950x; 3.5950x over previous
"""Trainium2 Bass kernel for an RNN-T style joint network MLP.

  out[b,t,u,o] = tanh(enc[b,t,:] @ W1[:512] + dec[b,u,:] @ W1[512:] + b1) @ W2 + b2

Shapes: enc (8, 256, 512), dec (8, 64, 512), W1 (1024, 1024), b1 (1024,),
W2 (1024, 128), b2 (128,), out (8, 256, 64, 128), all float32.

Sharding: data-parallel over batch - one batch element per NeuronCore, no
collectives. The kernel is ACT-bound (16.8M tanh/core at 1 elem/cycle/lane
@1.2GHz = 109us floor), so the design keeps ACT busy on (almost) nothing
but 16 full-width [128, 8192] tanh ops:
  - DVE: bias-broadcast adds sum[h,u,t] = e_proj[h,t] + bias[h,u]
    (tensor_scalar 2x mode, ~196ns per [128,256]) + most PSUM evacs (+b2).
  - ACT: the tanhs, plus every other block's pair-0 evac (load balance:
    DVE 6.27us adds + evac share vs ACT 7.01us tanh per block).
  - PE: first GEMMs + main GEMM (bf16 N=512 matmuls) + ~2.8us of junk
    warmup matmuls at t=0 so the HAM clock-gate reaches 2.4GHz before the
    real GEMMs arrive.
Head: inputs are host-rearranged so every DMA is contiguous; W1 is hc-major
and split into per-hc DMAs ordered across both HW DGE queues (sync+scalar)
to match first-use order; W2/b2 ride the slow gpsimd software queue. The
first block's tanh is quarter-split (a quarter needs only 2 h-chunks),
blocks 1-2 are half-split, and the last block is pair-split along u so
PE/DVE/DMA can chase the drain.
"""

import os
import numpy as np
import ml_dtypes

B, T, U, D, H, O = 8, 256, 64, 512, 1024, 128
NCORES = 8
HC = H // 128      # 8 h-chunks
UB = 4             # u-block size
NB = U // UB       # 16 blocks
BW = UB * T * HC   # 8192 per-block sum/tanh width (hc-major: [hc][u][t])
WARM_MMS = 26      # ~2.8us of junk matmuls to flip HAM to 2.4GHz

_CACHE = {}
LAST_RESULT = None


def _build_program():
    from concourse import bacc, tile
    import concourse.mybir as mybir

    dt = mybir.dt
    f32, bf16 = dt.float32, dt.bfloat16
    Act = mybir.ActivationFunctionType

    nc = bacc.Bacc("TRN2", target_bir_lowering=False, debug=False)

    encT = nc.dram_tensor("encT", [128, 4 * T], bf16, kind="ExternalInput").ap()
    decT = nc.dram_tensor("decT", [128, 4 * U], bf16, kind="ExternalInput").ap()
    w1r = nc.dram_tensor("w1r", [128, HC * 1024], bf16, kind="ExternalInput").ap()
    w2r = nc.dram_tensor("w2r", [128, HC * O], bf16, kind="ExternalInput").ap()
    b1r = nc.dram_tensor("b1r", [128, HC], f32, kind="ExternalInput").ap()
    b2c = nc.dram_tensor("b2c", [O, 1], f32, kind="ExternalInput").ap()
    outT = nc.dram_tensor("outT", [O, U, T], f32, kind="ExternalOutput").ap()

    with tile.TileContext(nc) as tc:
        with tc.tile_pool(name="persist", bufs=1) as persist, \
             tc.tile_pool(name="sums", bufs=3) as sums_pool, \
             tc.tile_pool(name="tanhp", bufs=3) as tanh_pool, \
             tc.tile_pool(name="outsb", bufs=6) as out_pool, \
             tc.tile_pool(name="psum", bufs=8, space="PSUM") as psum_pool:

            w1_sb = persist.tile([128, HC * 1024], bf16, tag="w1")
            encT_sb = persist.tile([128, 4 * T], bf16, tag="encT")
            decT_sb = persist.tile([128, 4 * U], bf16, tag="decT")
            w2_sb = persist.tile([128, HC * O], bf16, tag="w2")
            b1_sb = persist.tile([128, HC], f32, tag="b1")
            b2_sb = persist.tile([128, 1], f32, tag="b2")
            e_sb = persist.tile([128, HC * T], bf16, tag="eproj")
            bias_sb = persist.tile([128, HC * U], f32, tag="bias")
            junk_w = persist.tile([128, 128], bf16, tag="junkw")
            junk_r = persist.tile([128, 128], bf16, tag="junkr")
            warm_a = persist.tile([128, 128], bf16, tag="warma")
            warm_o = persist.tile([128, 128], bf16, tag="warmo")

            # --- input DMAs on the two HW DGE queues, first-use order ----
            # scalar queue (ACT issues, ~667ns each)
            nc.scalar.dma_start(decT_sb[:], decT[:, :])
            for hc in (0, 2, 5, 7):
                nc.scalar.dma_start(w1_sb[:, hc * 1024:(hc + 1) * 1024],
                                    w1r[:, hc * 1024:(hc + 1) * 1024])
            # sync queue (SP)
            nc.sync.dma_start(b1_sb[:], b1r[:, :])
            nc.sync.dma_start(encT_sb[:], encT[:, :])
            for hc in (1, 3, 4, 6):
                nc.sync.dma_start(w1_sb[:, hc * 1024:(hc + 1) * 1024],
                                  w1r[:, hc * 1024:(hc + 1) * 1024])
            # gpsimd software queue: slow (~2-3us descriptor gen) but these
            # are only needed ~12us in, and it keeps the HW queues clear
            nc.gpsimd.dma_start(w2_sb[:], w2r[:, :])
            nc.gpsimd.dma_start(b2_sb[:], b2c[:, :])

            # --- t=0 warmups ---------------------------------------------
            nc.vector.memset(junk_w[:], 0.0)
            nc.vector.memset(junk_r[:], 0.0)
            nc.vector.memset(warm_a[:], 0.0)
            # dummy tanh pulls the ~2.7us ACT_TABLE_LOAD off the critical
            # path (overlaps the input DMAs)
            nc.scalar.activation(warm_o[:], warm_a[:], Act.Tanh)
            # junk matmuls keep the HAM activity window busy so the real
            # first GEMM runs at 2.4GHz instead of 1.2
            for i in range(WARM_MMS):
                jp = psum_pool.tile([128, 128], f32, tag="ps", name=f"junk{i}")
                nc.tensor.matmul(jp[:], lhsT=junk_w[:], rhs=junk_r[:],
                                 start=True, stop=True)

            # --- head: first GEMMs per h-chunk, fused with block-0 adds --
            # w1 layout per hc: dcs 0-3 enc chunks, 4-7 dec chunks.
            sum0 = sums_pool.tile([128, BW], bf16, tag="sum")
            tanh0 = tanh_pool.tile([128, BW], bf16, tag="tanh")
            for hc in range(HC):
                pd = psum_pool.tile([128, U], f32, tag="ps", name=f"pd{hc}")
                for dc in range(4):
                    nc.tensor.matmul(
                        pd[:],
                        lhsT=w1_sb[:, hc * 1024 + (4 + dc) * 128:
                                   hc * 1024 + (4 + dc) * 128 + 128],
                        rhs=decT_sb[:, dc * U:(dc + 1) * U],
                        start=(dc == 0), stop=(dc == 3),
                    )
                # bias evac on ACT (cheap from PSUM; keeps DVE on adds)
                nc.scalar.activation(bias_sb[:, hc * U:(hc + 1) * U], pd[:],
                                     Act.Identity, bias=b1_sb[:, hc:hc + 1])

                pe = psum_pool.tile([128, T], f32, tag="ps", name=f"pe{hc}")
                for dc in range(4):
                    nc.tensor.matmul(
                        pe[:],
                        lhsT=w1_sb[:, hc * 1024 + dc * 128:
                                   hc * 1024 + dc * 128 + 128],
                        rhs=encT_sb[:, dc * T:(dc + 1) * T],
                        start=(dc == 0), stop=(dc == 3),
                    )
                nc.vector.tensor_copy(e_sb[:, hc * T:(hc + 1) * T], pe[:])

                # block-0 adds for this h-chunk
                for ul in range(UB):
                    nc.vector.tensor_scalar_add(
                        sum0[:, hc * (UB * T) + ul * T:
                             hc * (UB * T) + ul * T + T],
                        e_sb[:, hc * T:(hc + 1) * T],
                        bias_sb[:, hc * U + ul: hc * U + ul + 1])
                # quarter q of block 0's tanh needs only h-chunks 2q,2q+1
                if hc % 2 == 1:
                    q = hc // 2
                    nc.scalar.activation(
                        tanh0[:, q * BW // 4:(q + 1) * BW // 4],
                        sum0[:, q * BW // 4:(q + 1) * BW // 4], Act.Tanh)

            # --- steady loop over u-blocks --------------------------------
            for blk in range(NB):
                if blk == 0:
                    sum_sb, tanh_sb = sum0, tanh0
                else:
                    sum_sb = sums_pool.tile([128, BW], bf16, tag="sum")
                    for hc in range(HC):
                        for ul in range(UB):
                            u = blk * UB + ul
                            nc.vector.tensor_scalar_add(
                                sum_sb[:, hc * (UB * T) + ul * T:
                                       hc * (UB * T) + ul * T + T],
                                e_sb[:, hc * T:(hc + 1) * T],
                                bias_sb[:, hc * U + u: hc * U + u + 1])

                    tanh_sb = tanh_pool.tile([128, BW], bf16, tag="tanh")
                    if blk in (1, 2):
                        # half-split while the pipeline is still filling
                        for hq in range(2):
                            nc.scalar.activation(
                                tanh_sb[:, hq * BW // 2:(hq + 1) * BW // 2],
                                sum_sb[:, hq * BW // 2:(hq + 1) * BW // 2],
                                Act.Tanh)
                    elif blk == NB - 1:
                        # pair-split along u (strided [128, hc=8, 512] AP)
                        # so PE/DVE/DMA can chase the drain
                        sv = sum_sb[:, :].rearrange("p (c x) -> p c x", c=HC)
                        tv = tanh_sb[:, :].rearrange("p (c x) -> p c x", c=HC)
                        for p in range(2):
                            nc.scalar.activation(
                                tv[:, :, p * 512:(p + 1) * 512],
                                sv[:, :, p * 512:(p + 1) * 512], Act.Tanh)
                    else:
                        nc.scalar.activation(tanh_sb[:], sum_sb[:], Act.Tanh)

                # main GEMM: psum[o, pair cols] = sum_hc W2[hc]^T @ tanh[hc]
                pos = [psum_pool.tile([128, 512], f32, tag="ps",
                                      name=f"po{blk}_{p}") for p in range(2)]
                if blk == NB - 1:
                    # pair-outer so pair 0 finishes right after its tanh
                    for p in range(2):
                        for hc in range(HC):
                            nc.tensor.matmul(
                                pos[p][:],
                                lhsT=w2_sb[:, hc * O:(hc + 1) * O],
                                rhs=tanh_sb[:, hc * (UB * T) + p * 512:
                                            hc * (UB * T) + (p + 1) * 512],
                                start=(hc == 0), stop=(hc == HC - 1),
                            )
                        osb = out_pool.tile([128, 512], f32, tag="osb",
                                            name=f"ot{p}")
                        nc.vector.tensor_scalar_add(osb[:], pos[p][:],
                                                    b2_sb[:, 0:1])
                        nc.sync.dma_start(
                            outT[:, blk * UB + 2 * p: blk * UB + 2 * p + 2, :],
                            osb[:])
                else:
                    for hc in range(HC):
                        for p in range(2):
                            nc.tensor.matmul(
                                pos[p][:],
                                lhsT=w2_sb[:, hc * O:(hc + 1) * O],
                                rhs=tanh_sb[:, hc * (UB * T) + p * 512:
                                            hc * (UB * T) + (p + 1) * 512],
                                start=(hc == 0), stop=(hc == HC - 1),
                            )
                    for p in range(2):
                        osb = out_pool.tile([128, 512], f32, tag="osb",
                                            name=f"o{blk}_{p}")
                        # balance: odd blocks' pair-0 evac goes to ACT
                        if p == 0 and blk % 2 == 1:
                            nc.scalar.activation(osb[:], pos[p][:],
                                                 Act.Identity,
                                                 bias=b2_sb[:, 0:1])
                        else:
                            nc.vector.tensor_scalar_add(osb[:], pos[p][:],
                                                        b2_sb[:, 0:1])
                        nc.sync.dma_start(
                            outT[:, blk * UB + 2 * p: blk * UB + 2 * p + 2, :],
                            osb[:])

    nc.compile()
    return nc


def kernel(encoder_state, decoder_state, W1, b1, W2, b2):
    from concourse.bass_utils import run_bass_kernel_spmd
    global LAST_RESULT

    if "nc" not in _CACHE:
        _CACHE["nc"] = _build_program()
    nc = _CACHE["nc"]

    encoder_state = np.asarray(encoder_state, dtype=np.float32)
    decoder_state = np.asarray(decoder_state, dtype=np.float32)
    W1 = np.asarray(W1, dtype=np.float32)
    b1 = np.asarray(b1, dtype=np.float32)
    W2 = np.asarray(W2, dtype=np.float32)
    b2 = np.asarray(b2, dtype=np.float32)

    bf = ml_dtypes.bfloat16
    # W1 [2D, H] -> hc-major [128, hc(8), dcs(8), 128]: per hc, 4 enc then
    # 4 dec contraction chunks, each [128 x 128] with d on partitions.
    W1r = np.ascontiguousarray(
        W1.astype(bf).reshape(8, 128, 8, 128).transpose(1, 2, 0, 3)
        .reshape(128, 8192))
    # W2 [H, O] -> [128, hc(8)*O] with h-within-chunk on partitions
    W2r = np.ascontiguousarray(
        W2.astype(bf).reshape(8, 128, O).transpose(1, 0, 2).reshape(128, 8 * O))
    b1r = np.ascontiguousarray(b1.reshape(HC, 128).T)
    b2c = np.ascontiguousarray(b2.reshape(O, 1))

    in_maps = []
    for i in range(NCORES):
        encT = np.ascontiguousarray(
            encoder_state[i].T.astype(bf).reshape(4, 128, T)
            .transpose(1, 0, 2).reshape(128, 4 * T))
        decT = np.ascontiguousarray(
            decoder_state[i].T.astype(bf).reshape(4, 128, U)
            .transpose(1, 0, 2).reshape(128, 4 * U))
        in_maps.append({
            "encT": encT,
            "decT": decT,
            "w1r": W1r,
            "w2r": W2r,
            "b1r": b1r,
            "b2c": b2c,
        })

    trace = bool(int(os.environ.get("KERNEL_TRACE", "0")))
    res = run_bass_kernel_spmd(nc, in_maps, list(range(NCORES)), trace=trace)
    LAST_RESULT = res

    out = np.empty((B, T, U, O), dtype=np.float32)
    for i in range(NCORES):
        out[i] = res.results[i]["outT"].transpose(2, 1, 0)
    return out
